# revision 12
# baseline (speedup 1.0000x reference)
"""Trainium2 Bass kernel for nn_CoverageLoss (retrieval_knn).

Math reduction: the reference loss only needs, per space sample s, the 4
smallest L1 distances to all latents (TAIL == PUSH == 4, and the phase-2
"close" values for the FAR rows are exactly those rows' 4 smallest
distances).  So each core computes, for its latent shard [N/8, 64], the 8
smallest exact L1 distances per space sample (Max8 on negated sums), and the
host merges the 8x8 per-row candidates, takes the global 4 smallest, and
finishes the tiny reduction (tail means -> top-64 rows -> Huber mean).

Device algorithm per core, using |b-a| = 2*max(a,b) - a - b so every DVE op
is a plain max/add (abs is not an ISA-valid tensor_scalar op1):
  - B_T2 [128, NLOC] bf16: transposed latent shard stacked twice
    (rows 0..63 = latT, rows 64..127 = latT again).
  - Per pair of space samples (s0, s1): M[k, n] = max(b(n, d(k)), a(s(k), d(k)))
    with k = s_local*64 + d, via one tensor_scalar_max (DVE 4x bf16 mode).
  - The PE contracts over k with a constant stationary [128, 32] holding -2
    at (k<64, col 2t) and (k>=64, col 2t+1): 16 pairs accumulate into one
    PSUM quadrant [32, 512], so a full [128, 512] PSUM bank holds
    -2*sum_d max for 128 consecutive samples x 512 latents.
  - Eviction adds the corrections: rowbuf = psum + rowsumA(s) + rowsumB(n)
    (= -L1 distance) via one scalar_tensor_tensor(add, add) per chunk.
  - DVE Max8 extracts the 8 largest per row (= 8 smallest distances).
"""

import numpy as np
import ml_dtypes
from contextlib import ExitStack

S = 2048
N = 65536
D = 64
NCORES = 8
NLOC = N // NCORES  # 8192
CHUNK = 512  # matmul moving free dim / PSUM bank columns
T_COLS = 4096  # columns per max-tile (8 chunks)

_cache = {}


def _build(nloc, s):
    import concourse.tile as tile
    from concourse import bacc, mybir

    nc = bacc.Bacc(
        "TRN2",
        target_bir_lowering=False,
        debug=False,
        num_devices=NCORES,
    )
    f32 = mybir.dt.float32
    bf16 = mybir.dt.bfloat16

    lat_t = nc.dram_tensor("latT", [D, nloc], bf16, kind="ExternalInput").ap()
    a_col = nc.dram_tensor("aCol", [128, s // 2], f32, kind="ExternalInput").ap()
    a_sum = nc.dram_tensor("aSum", [128, s // 128], f32, kind="ExternalInput").ap()
    b_sum = nc.dram_tensor("bSum", [1, nloc], f32, kind="ExternalInput").ap()
    tails = nc.dram_tensor("tails", [s, 8], f32, kind="ExternalOutput").ap()

    n_sblocks = s // 128
    t_cols = min(T_COLS, nloc)
    n_tgroups = nloc // t_cols
    chunks_per_t = t_cols // CHUNK

    with tile.TileContext(nc) as tc, ExitStack() as ctx:
        const_pool = ctx.enter_context(tc.tile_pool(name="const", bufs=1))
        t_pool = ctx.enter_context(tc.tile_pool(name="ttile", bufs=6))
        psum_pool = ctx.enter_context(
            tc.tile_pool(name="psum", bufs=8, space="PSUM")
        )
        row_pool = ctx.enter_context(tc.tile_pool(name="rowbuf", bufs=2))
        out_pool = ctx.enter_context(tc.tile_pool(name="outs", bufs=2))

        # Per-pair sample columns [128, s//2] and per-block rowsum columns
        # (loaded first: the first max-tile only needs na + bt2's first group).
        na = const_pool.tile([128, s // 2], f32)
        nc.sync.dma_start(na[:, :], a_col[:, :])

        # Transposed latents, stacked twice along partitions.
        bt2 = const_pool.tile([128, nloc], bf16)
        nc.sync.dma_start(bt2[0:64, :], lat_t[:, :])
        nc.sync.dma_start(bt2[64:128, :], lat_t[:, :])

        asum = const_pool.tile([128, s // 128], f32)
        nc.sync.dma_start(asum[:, :], a_sum[:, :])

        # Latent row-sums broadcast to all partitions (gpsimd queue: only
        # gates evictions, keep it off the sync queue's critical path).
        bsum = const_pool.tile([128, nloc], f32)
        nc.gpsimd.dma_start(bsum[:, :], b_sum[0:1, :].to_broadcast([128, nloc]))

        # 16 stationary tiles [128, 32]: tile t has -2 at (k<64, 2t) and
        # (k>=64, 2t+1) - the -2 of the max identity, negated so Max8 finds
        # the smallest distances.
        ones = const_pool.tile([128, 16 * 32], bf16)
        nc.vector.memset(ones[:, :], 0.0)
        for t in range(16):
            nc.vector.memset(ones[0:64, 32 * t + 2 * t : 32 * t + 2 * t + 1], -2.0)
            nc.vector.memset(
                ones[64:128, 32 * t + 2 * t + 1 : 32 * t + 2 * t + 2], -2.0
            )

        # Max8 is split per t-group: the first half runs eagerly mid-block,
        # the last half + merge are deferred into the middle of the next
        # block's production, so no DVE burst ever exceeds the PE's T-tile
        # backlog and the PE never stalls at block boundaries.
        pending = []  # [(rowbuf, maxh, sb)]

        def flush_pending():
            while pending:
                rb, mh, psb = pending.pop()
                lo = (n_tgroups - 1) * 8
                nc.vector.max(
                    out=mh[:, lo : lo + 8],
                    in_=rb[:, (n_tgroups - 1) * t_cols : n_tgroups * t_cols],
                )
                if n_tgroups > 1:
                    maxb = out_pool.tile([128, 8], f32, name="maxb")
                    nc.vector.max(out=maxb[:, :], in_=mh[:, :])
                else:
                    maxb = mh
                nc.sync.dma_start(
                    tails[psb * 128 : (psb + 1) * 128, :], maxb[:, :8]
                )

        for sb in range(n_sblocks):
            rowbuf = row_pool.tile([128, nloc], f32)
            maxh = out_pool.tile([128, 8 * n_tgroups], f32, name="maxh")
            for tg in range(n_tgroups):
                if tg == n_tgroups - 1:
                    flush_pending()
                psums = [
                    psum_pool.tile(
                        [128, CHUNK], f32, space="PSUM", tag="psumb", name="psumb"
                    )
                    for _ in range(chunks_per_t)
                ]
                for jj in range(64):  # s-pairs within this 128-sample block
                    pair = sb * 64 + jj
                    q, t = divmod(jj, 16)
                    ttile = t_pool.tile([128, t_cols], bf16)
                    nc.vector.tensor_scalar_max(
                        ttile[:, :],
                        bt2[:, tg * t_cols : (tg + 1) * t_cols],
                        na[:, pair : pair + 1],
                    )
                    for c in range(chunks_per_t):
                        nc.tensor.matmul(
                            psums[c][32 * q : 32 * (q + 1), :],
                            ones[:, 32 * t : 32 * (t + 1)],
                            ttile[:, c * CHUNK : (c + 1) * CHUNK],
                            start=(t == 0),
                            stop=(t == 15),
                            tile_position=(0, 32 * q),
                        )
                for c in range(chunks_per_t):
                    col = (tg * chunks_per_t + c) * CHUNK
                    nc.vector.scalar_tensor_tensor(
                        rowbuf[:, col : col + CHUNK],
                        psums[c][:, :],
                        asum[:, sb : sb + 1],
                        bsum[:, col : col + CHUNK],
                        op0=mybir.AluOpType.add,
                        op1=mybir.AluOpType.add,
                    )
                if tg < n_tgroups - 1:
                    nc.vector.max(
                        out=maxh[:, tg * 8 : (tg + 1) * 8],
                        in_=rowbuf[:, tg * t_cols : (tg + 1) * t_cols],
                    )
            pending.append((rowbuf, maxh, sb))
        flush_pending()

    nc.compile()
    return nc


def _get_nc(nloc=NLOC, s=S):
    key = (nloc, s)
    if key not in _cache:
        _cache[key] = _build(nloc, s)
    return _cache[key]


def _prep_inputs(latents, space_samples, nloc=NLOC):
    latents = np.asarray(latents, dtype=np.float32)
    ss = np.asarray(space_samples, dtype=np.float32)
    s = ss.shape[0]
    lat_t = np.ascontiguousarray(latents.T).astype(ml_dtypes.bfloat16)  # [64, N]
    # aCol[p*64 + d, j] = ss[2j + p, d]  for p in {0,1}
    a_col = np.ascontiguousarray(
        ss.reshape(s // 2, 2, D).transpose(1, 2, 0).reshape(128, s // 2)
    ).astype(np.float32)
    # aSum[p, sb] = sum_d ss[sb*128 + p, d]
    a_sum = np.ascontiguousarray(ss.sum(axis=1).reshape(s // 128, 128).T).astype(
        np.float32
    )
    b_sum_full = latents.sum(axis=1).astype(np.float32)  # [N]
    ncores = latents.shape[0] // nloc
    in_maps = [
        {
            "latT": np.ascontiguousarray(lat_t[:, c * nloc : (c + 1) * nloc]),
            "aCol": a_col,
            "aSum": a_sum,
            "bSum": np.ascontiguousarray(b_sum_full[c * nloc : (c + 1) * nloc])[
                None, :
            ],
        }
        for c in range(ncores)
    ]
    return in_maps


def _finish(per_core_tails, space_samples):
    """per_core_tails: [ncores, S, 8] Max8 outputs of negated distances."""
    ss = np.asarray(space_samples, dtype=np.float32)
    s = ss.shape[0]
    cand = -np.concatenate(list(per_core_tails), axis=1).reshape(s, -1)
    cand.sort(axis=1)
    tail = cand[:, :4]  # [S, 4] smallest distances, ascending
    tail_mean = tail.mean(axis=1)
    far = np.argsort(-tail_mean, kind="stable")[:64]
    close = cand[far][:, :4]
    a = np.abs(close)
    huber = np.where(a <= 1.0, 0.5 * close * close, a - 0.5)
    return np.float32(huber.mean())


def _run_device(latents, space_samples, trace=False):
    from concourse.bass_utils import run_bass_kernel_spmd

    nc = _get_nc()
    in_maps = _prep_inputs(latents, space_samples)
    res = run_bass_kernel_spmd(nc, in_maps, list(range(NCORES)), trace=trace)
    tails = np.stack([res.results[c]["tails"] for c in range(NCORES)])
    return tails, res


def kernel(latents, space_samples):
    tails, _ = _run_device(latents, space_samples, trace=False)
    return _finish(tails, space_samples)


def run_traced(latents, space_samples):
    """Like kernel() but with NTFF profiling; returns (loss, exec_time_ns)."""
    tails, res = _run_device(latents, space_samples, trace=True)
    return _finish(tails, space_samples), res.exec_time_ns


# revision 16
# speedup vs baseline: 1.0060x; 1.0060x over previous
"""Trainium2 Bass kernel for nn_CoverageLoss (retrieval_knn).

Math reduction: the reference loss only needs, per space sample s, the 4
smallest L1 distances to all latents (TAIL == PUSH == 4, and the phase-2
"close" values for the FAR rows are exactly those rows' 4 smallest
distances).  So each core computes, for its latent shard [N/8, 64], the 8
smallest exact L1 distances per space sample (Max8 on negated sums), and the
host merges the 8x8 per-row candidates, takes the global 4 smallest, and
finishes the tiny reduction (tail means -> top-64 rows -> Huber mean).

Device algorithm per core, using |b-a| = 2*max(a,b) - a - b so every DVE op
is a plain max/add (abs is not an ISA-valid tensor_scalar op1):
  - B_T2 [128, NLOC] bf16: transposed latent shard stacked twice
    (rows 0..63 = latT, rows 64..127 = latT again).
  - Per pair of space samples (s0, s1): M[k, n] = max(b(n, d(k)), a(s(k), d(k)))
    with k = s_local*64 + d, via one tensor_scalar_max (DVE 4x bf16 mode).
  - The PE contracts over k with a constant stationary [128, 32] holding -2
    at (k<64, col 2t) and (k>=64, col 2t+1): 16 pairs accumulate into one
    PSUM quadrant [32, 512], so a full [128, 512] PSUM bank holds
    -2*sum_d max for 128 consecutive samples x 512 latents.
  - Eviction adds the corrections: rowbuf = psum + rowsumA(s) + rowsumB(n)
    (= -L1 distance) via one scalar_tensor_tensor(add, add) per chunk.
  - DVE Max8 extracts the 8 largest per row (= 8 smallest distances).
"""

import numpy as np
import ml_dtypes
from contextlib import ExitStack

S = 2048
N = 65536
D = 64
NCORES = 8
NLOC = N // NCORES  # 8192
CHUNK = 512  # matmul moving free dim / PSUM bank columns
T_COLS = 4096  # columns per max-tile (8 chunks)

_cache = {}


def _build(nloc, s):
    import concourse.tile as tile
    from concourse import bacc, mybir

    nc = bacc.Bacc(
        "TRN2",
        target_bir_lowering=False,
        debug=False,
        num_devices=NCORES,
    )
    f32 = mybir.dt.float32
    bf16 = mybir.dt.bfloat16

    lat_t = nc.dram_tensor("latT", [D, nloc], bf16, kind="ExternalInput").ap()
    a_col = nc.dram_tensor("aCol", [128, s // 2], f32, kind="ExternalInput").ap()
    a_sum = nc.dram_tensor("aSum", [128, s // 128], f32, kind="ExternalInput").ap()
    b_sum = nc.dram_tensor("bSum", [1, nloc], f32, kind="ExternalInput").ap()
    tails = nc.dram_tensor("tails", [s, 8], f32, kind="ExternalOutput").ap()

    n_sblocks = s // 128
    t_cols = min(T_COLS, nloc)
    n_tgroups = nloc // t_cols
    chunks_per_t = t_cols // CHUNK

    with tile.TileContext(nc) as tc, ExitStack() as ctx:
        const_pool = ctx.enter_context(tc.tile_pool(name="const", bufs=1))
        t_pool = ctx.enter_context(tc.tile_pool(name="ttile", bufs=7))
        psum_pool = ctx.enter_context(
            tc.tile_pool(name="psum", bufs=8, space="PSUM")
        )
        row_pool = ctx.enter_context(tc.tile_pool(name="rowbuf", bufs=2))
        out_pool = ctx.enter_context(tc.tile_pool(name="outs", bufs=2))

        # Per-pair sample columns [128, s//2] and per-block rowsum columns
        # (loaded first: the first max-tile only needs na + bt2's first group).
        na = const_pool.tile([128, s // 2], f32)
        nc.sync.dma_start(na[:, :], a_col[:, :])

        # Transposed latents, stacked twice along partitions; chunked per
        # t-group so production can start before the whole shard lands.
        bt2 = const_pool.tile([128, nloc], bf16)
        for tg in range(n_tgroups):
            cols = slice(tg * t_cols, (tg + 1) * t_cols)
            nc.sync.dma_start(bt2[0:64, cols], lat_t[:, cols])
            nc.sync.dma_start(bt2[64:128, cols], lat_t[:, cols])

        asum = const_pool.tile([128, s // 128], f32)
        nc.sync.dma_start(asum[:, :], a_sum[:, :])

        # Latent row-sums broadcast to all partitions (gpsimd queue: only
        # gates evictions, keep it off the sync queue's critical path).
        bsum = const_pool.tile([128, nloc], f32)
        nc.gpsimd.dma_start(bsum[:, :], b_sum[0:1, :].to_broadcast([128, nloc]))

        # 16 stationary tiles [128, 32]: tile t has -2 at (k<64, 2t) and
        # (k>=64, 2t+1) - the -2 of the max identity, negated so Max8 finds
        # the smallest distances.
        ones = const_pool.tile([128, 16 * 32], bf16)
        nc.vector.memset(ones[:, :], 0.0)
        for t in range(16):
            nc.vector.memset(ones[0:64, 32 * t + 2 * t : 32 * t + 2 * t + 1], -2.0)
            nc.vector.memset(
                ones[64:128, 32 * t + 2 * t + 1 : 32 * t + 2 * t + 2], -2.0
            )

        # Warm the PE while the input DMAs land: the HAM clock gate keeps an
        # idle PE at 1.2 GHz and needs ~3.4us of sustained activity to open
        # to 2.4 GHz, so burn dummy matmuls on the ones tile into a scratch
        # PSUM tile that nothing reads.
        warm = psum_pool.tile(
            [128, CHUNK], f32, space="PSUM", tag="psumb", name="psumb"
        )
        for _ in range(20):
            nc.tensor.matmul(
                warm[0:32, :],
                ones[:, 0:32],
                ones[:, 0:CHUNK],
                start=True,
                stop=True,
                tile_position=(0, 0),
            )

        # Max8 is split per t-group: the first half runs eagerly mid-block,
        # the last half + merge are deferred into the middle of the next
        # block's production, so no DVE burst ever exceeds the PE's T-tile
        # backlog and the PE never stalls at block boundaries.
        pending = []  # [(rowbuf, maxh, sb)]

        def flush_pending():
            while pending:
                rb, mh, psb = pending.pop()
                lo = (n_tgroups - 1) * 8
                nc.vector.max(
                    out=mh[:, lo : lo + 8],
                    in_=rb[:, (n_tgroups - 1) * t_cols : n_tgroups * t_cols],
                )
                if n_tgroups > 1:
                    maxb = out_pool.tile([128, 8], f32, name="maxb")
                    nc.vector.max(out=maxb[:, :], in_=mh[:, :])
                else:
                    maxb = mh
                nc.sync.dma_start(
                    tails[psb * 128 : (psb + 1) * 128, :], maxb[:, :8]
                )

        for sb in range(n_sblocks):
            rowbuf = row_pool.tile([128, nloc], f32)
            maxh = out_pool.tile([128, 8 * n_tgroups], f32, name="maxh")
            for tg in range(n_tgroups):
                if tg == n_tgroups - 1:
                    flush_pending()
                psums = [
                    psum_pool.tile(
                        [128, CHUNK], f32, space="PSUM", tag="psumb", name="psumb"
                    )
                    for _ in range(chunks_per_t)
                ]
                for jj in range(64):  # s-pairs within this 128-sample block
                    pair = sb * 64 + jj
                    q, t = divmod(jj, 16)
                    ttile = t_pool.tile([128, t_cols], bf16)
                    nc.vector.tensor_scalar_max(
                        ttile[:, :],
                        bt2[:, tg * t_cols : (tg + 1) * t_cols],
                        na[:, pair : pair + 1],
                    )
                    for c in range(chunks_per_t):
                        nc.tensor.matmul(
                            psums[c][32 * q : 32 * (q + 1), :],
                            ones[:, 32 * t : 32 * (t + 1)],
                            ttile[:, c * CHUNK : (c + 1) * CHUNK],
                            start=(t == 0),
                            stop=(t == 15),
                            tile_position=(0, 32 * q),
                        )
                for c in range(chunks_per_t):
                    col = (tg * chunks_per_t + c) * CHUNK
                    nc.vector.scalar_tensor_tensor(
                        rowbuf[:, col : col + CHUNK],
                        psums[c][:, :],
                        asum[:, sb : sb + 1],
                        bsum[:, col : col + CHUNK],
                        op0=mybir.AluOpType.add,
                        op1=mybir.AluOpType.add,
                    )
                if tg < n_tgroups - 1:
                    nc.vector.max(
                        out=maxh[:, tg * 8 : (tg + 1) * 8],
                        in_=rowbuf[:, tg * t_cols : (tg + 1) * t_cols],
                    )
            pending.append((rowbuf, maxh, sb))
        flush_pending()

    nc.compile()
    return nc


def _get_nc(nloc=NLOC, s=S):
    key = (nloc, s)
    if key not in _cache:
        _cache[key] = _build(nloc, s)
    return _cache[key]


def _prep_inputs(latents, space_samples, nloc=NLOC):
    latents = np.asarray(latents, dtype=np.float32)
    ss = np.asarray(space_samples, dtype=np.float32)
    s = ss.shape[0]
    lat_t = np.ascontiguousarray(latents.T).astype(ml_dtypes.bfloat16)  # [64, N]
    # aCol[p*64 + d, j] = ss[2j + p, d]  for p in {0,1}
    a_col = np.ascontiguousarray(
        ss.reshape(s // 2, 2, D).transpose(1, 2, 0).reshape(128, s // 2)
    ).astype(np.float32)
    # aSum[p, sb] = sum_d ss[sb*128 + p, d]
    a_sum = np.ascontiguousarray(ss.sum(axis=1).reshape(s // 128, 128).T).astype(
        np.float32
    )
    b_sum_full = latents.sum(axis=1).astype(np.float32)  # [N]
    ncores = latents.shape[0] // nloc
    in_maps = [
        {
            "latT": np.ascontiguousarray(lat_t[:, c * nloc : (c + 1) * nloc]),
            "aCol": a_col,
            "aSum": a_sum,
            "bSum": np.ascontiguousarray(b_sum_full[c * nloc : (c + 1) * nloc])[
                None, :
            ],
        }
        for c in range(ncores)
    ]
    return in_maps


def _finish(per_core_tails, space_samples):
    """per_core_tails: [ncores, S, 8] Max8 outputs of negated distances."""
    ss = np.asarray(space_samples, dtype=np.float32)
    s = ss.shape[0]
    cand = -np.concatenate(list(per_core_tails), axis=1).reshape(s, -1)
    cand.sort(axis=1)
    tail = cand[:, :4]  # [S, 4] smallest distances, ascending
    tail_mean = tail.mean(axis=1)
    far = np.argsort(-tail_mean, kind="stable")[:64]
    close = cand[far][:, :4]
    a = np.abs(close)
    huber = np.where(a <= 1.0, 0.5 * close * close, a - 0.5)
    return np.float32(huber.mean())


def _run_device(latents, space_samples, trace=False):
    from concourse.bass_utils import run_bass_kernel_spmd

    nc = _get_nc()
    in_maps = _prep_inputs(latents, space_samples)
    res = run_bass_kernel_spmd(nc, in_maps, list(range(NCORES)), trace=trace)
    tails = np.stack([res.results[c]["tails"] for c in range(NCORES)])
    return tails, res


def kernel(latents, space_samples):
    tails, _ = _run_device(latents, space_samples, trace=False)
    return _finish(tails, space_samples)


def run_traced(latents, space_samples):
    """Like kernel() but with NTFF profiling; returns (loss, exec_time_ns)."""
    tails, res = _run_device(latents, space_samples, trace=True)
    return _finish(tails, space_samples), res.exec_time_ns
